# revision 1
# baseline (speedup 1.0000x reference)
"""DFlashAttention Trainium2 kernel (8-core tensor-parallel over attention heads).

Shapes (hardcoded): D=2048, N=16 q-heads, K=8 kv-heads, H=128,
T_NOISE=2048 (query tokens), T_CTX=4096, S=6144 (kv tokens).

Sharding: core c owns q-heads {2c, 2c+1} and kv-head c (GQA groups=2).
Each core computes a partial (T, D) output (its 2 heads' slice of the
o-projection contraction); the host sums the 8 partials (TP unshard).

Layout strategy per core:
  - x_all^T [D, S] fed replicated (d on partitions = matmul contraction dim).
  - kv proj:  psum[s,0:128]=k, psum[s,128:256]=v  (one fp32r matmul chain,
    moving free dim 256).
  - RMSNorm over H via ACT Square+accum_out; RoPE via on-device sin/cos
    (angle mod 2pi + range wrap + ACT Sin); tables built once for all 48
    token tiles.
  - attention in [s, t] orientation: scores^T = kT.T @ qT (contraction H=128,
    single matmul per (s-tile, t-chunk)); exp on ACT (scale=1/sqrt(H) folded);
    no max subtraction (|score| <= sqrt(H)*1.1^2 ~ 13.7 after RMSNorm, exp is
    safe in fp32); row-sums via ones-matmul; A@V accumulates over s-tiles in
    PSUM with V in natural [s, h] layout.
  - softmax division deferred past the o-projection (denominator is constant
    along the contraction), where it is a per-partition scalar multiply.
"""

import sys

for _p in ("/opt/trn_rl_repo", "/root/.axon_site/_ro/trn_rl_repo"):
    if _p not in sys.path:
        sys.path.append(_p)

import math
import numpy as np

import concourse.bass as bass
import concourse.tile as tile
from concourse import bacc
from concourse import mybir
from concourse.bass_utils import run_bass_kernel_spmd
from concourse.masks import make_identity

D = 2048
N_HEADS = 16
K_HEADS = 8
H = 128
T_NOISE = 2048
T_CTX = 4096
S_ALL = T_CTX + T_NOISE          # 6144
EPS = 1e-6
ROPE_THETA = 1e6
N_CORES = 8
HEADS_PER_CORE = N_HEADS // N_CORES   # 2

P = 128                       # partition dim
S_TILES = S_ALL // P          # 48
T_TILES = T_NOISE // P        # 16
NOISE_TILE0 = T_CTX // P      # 32  (noise tokens are s-tiles 32..47)
D_TILES = D // P              # 16
FREE = 512                    # moving free-dim chunk
T_CHUNKS = T_NOISE // FREE    # 4
S_CHUNKS = S_ALL // FREE      # 12
D_CHUNKS = D // FREE          # 4

F32 = mybir.dt.float32
F32R = mybir.dt.float32r
MM_DT = F32R                  # dtype for all matmul operands

TWO_PI = 2.0 * math.pi
INV_SQRT_H = 1.0 / math.sqrt(H)

_CACHE = {}


def _build_program(reps=1):
    """Build the single-core SPMD bass program. Returns (nc, out_name).
    reps>1 repeats the whole kernel body (timing harness only)."""
    nc = bacc.Bacc("TRN2", target_bir_lowering=False, debug=False,
                   num_devices=N_CORES)

    xT = nc.dram_tensor("xT", [D, S_ALL], MM_DT, kind="ExternalInput").ap()
    wkv = nc.dram_tensor("wkv", [D, 2 * H], MM_DT, kind="ExternalInput").ap()
    wq = nc.dram_tensor("wq", [D, HEADS_PER_CORE * H], MM_DT,
                        kind="ExternalInput").ap()
    wo = nc.dram_tensor("wo", [HEADS_PER_CORE, H, D], MM_DT,
                        kind="ExternalInput").ap()
    posr = nc.dram_tensor("posr", [S_TILES, P, 1], F32,
                          kind="ExternalInput").ap()
    invfb = nc.dram_tensor("invfb", [P, H // 2], F32,
                           kind="ExternalInput").ap()
    qscaleb = nc.dram_tensor("qscaleb", [P, H], F32,
                             kind="ExternalInput").ap()
    kscaleb = nc.dram_tensor("kscaleb", [P, H], F32,
                             kind="ExternalInput").ap()
    onesb = nc.dram_tensor("onesb", [P, 1], MM_DT, kind="ExternalInput").ap()
    out = nc.dram_tensor("out", [T_NOISE, D], F32, kind="ExternalOutput").ap()

    with tile.TileContext(nc) as tc:
        for rep in range(reps):
            _emit(nc, tc, xT, wkv, wq, wo, posr, invfb, qscaleb, kscaleb,
                  onesb, out, pfx=f"r{rep}_")
    nc.compile()
    return nc, "out"


def _emit(nc, tc, xT, wkv, wq, wo, posr, invfb, qscaleb, kscaleb, onesb, out, pfx=""):
    import contextlib
    ctx = contextlib.ExitStack()
    with ctx:
        const = ctx.enter_context(tc.tile_pool(name=pfx + "const", bufs=1))
        persist = ctx.enter_context(tc.tile_pool(name=pfx + "persist", bufs=1))

        # ---- constants ----
        ident = const.tile([P, P], F32, tag="ident")
        make_identity(nc, ident[:])
        ones = const.tile([P, 1], MM_DT, tag="ones")
        nc.sync.dma_start(ones[:], onesb[:])
        invf_sb = const.tile([P, H // 2], F32, tag="invf")
        nc.sync.dma_start(invf_sb[:], invfb[:])
        qsc_sb = const.tile([P, H], F32, tag="qsc")
        nc.sync.dma_start(qsc_sb[:], qscaleb[:])
        ksc_sb = const.tile([P, H], F32, tag="ksc")
        nc.sync.dma_start(ksc_sb[:], kscaleb[:])
        eps_col = const.tile([P, 1], F32, tag="eps")
        nc.vector.memset(eps_col[:], EPS)
        pos_sb = const.tile([P, S_TILES], F32, tag="pos")
        for si in range(S_TILES):
            nc.sync.dma_start(pos_sb[:, si:si + 1], posr[si])

        wkv_sb = [const.tile([P, 2 * H], MM_DT, tag=f"wkv{d}", name=f"wkv{d}")
                  for d in range(D_TILES)]
        wq_sb = [const.tile([P, HEADS_PER_CORE * H], MM_DT, tag=f"wq{d}", name=f"wqs{d}")
                 for d in range(D_TILES)]
        for d in range(D_TILES):
            nc.sync.dma_start(wkv_sb[d][:], wkv[d * P:(d + 1) * P, :])
            nc.sync.dma_start(wq_sb[d][:], wq[d * P:(d + 1) * P, :])
        wo_sb = [const.tile([P, D], MM_DT, tag=f"wo{h}", name=f"wos{h}")
                 for h in range(HEADS_PER_CORE)]
        for h in range(HEADS_PER_CORE):
            nc.sync.dma_start(wo_sb[h][:], wo[h])

        # ---- persistent activations ----
        half = H // 2
        sin_all = persist.tile([P, S_TILES * half], F32, tag="sin")
        cos_all = persist.tile([P, S_TILES * half], F32, tag="cos")
        kT_sb = persist.tile([P, S_ALL], MM_DT, tag="kT")
        v_sb = persist.tile([P, S_ALL], MM_DT, tag="v")       # [s-tile, h] blocks
        qT_sb = persist.tile([P, HEADS_PER_CORE * T_NOISE], MM_DT, tag="qT")
        oT_sb = persist.tile([P, HEADS_PER_CORE * T_NOISE], MM_DT, tag="oT")
        r_all = persist.tile([1, HEADS_PER_CORE * T_NOISE], F32, tag="r")
        rcol = persist.tile([P, HEADS_PER_CORE * T_TILES], F32, tag="rcol")

        # ---- RoPE sin/cos tables for all 48 token tiles ----
        # angle = pos * inv_freq; range-reduce mod 2pi via Cody-Waite
        # (k = int(angle/2pi); red = ((ang - k*c1) - k*c2) - k*c3).
        CW1, CW2, CW3 = 6.28125, 0.0019353071693331003, 1.0253131677018246e-11
        HGRP = S_TILES // 2
        HW_ = HGRP * half
        with tc.tile_pool(name=pfx + "ropebuild", bufs=1) as rp:
            for g in range(2):
                ang = rp.tile([P, HW_], F32, tag="ang", name="ang")
                kq = rp.tile([P, HW_], F32, tag="kq", name="kq")
                ki = rp.tile([P, HW_], mybir.dt.int32, tag="ki", name="ki")
                wrap = rp.tile([P, HW_], F32, tag="wrap", name="wrap")
                for j in range(HGRP):
                    si = g * HGRP + j
                    nc.vector.tensor_scalar_mul(
                        ang[:, j * half:(j + 1) * half], invf_sb[:, :],
                        pos_sb[:, si:si + 1])
                nc.vector.tensor_scalar_mul(kq[:], ang[:], 1.0 / TWO_PI)
                nc.vector.tensor_copy(ki[:], kq[:])
                nc.vector.tensor_copy(kq[:], ki[:])
                nc.vector.cody_waite_cascade(ang[:], ang[:], kq[:],
                                             CW1, CW2, CW3)
                dst = slice(g * HW_, (g + 1) * HW_)
                nc.vector.add_range_wrap(wrap[:], ang[:], 0.0, math.pi, TWO_PI)
                nc.scalar.activation(sin_all[:, dst], wrap[:],
                                     mybir.ActivationFunctionType.Sin)
                nc.vector.add_range_wrap(wrap[:], ang[:], math.pi / 2, math.pi,
                                         TWO_PI)
                nc.scalar.activation(cos_all[:, dst], wrap[:],
                                     mybir.ActivationFunctionType.Sin)

        def norm_rope_transpose(src_psum, scale_sb, si, dst_sb, work, psum_t):
            """src_psum [P(tok),H] fp32 -> rms-norm*scale -> rope -> transpose
            -> dst_sb [P(h), 128 tok]. si = token-tile index for positions."""
            sq = work.tile([P, H], F32, tag="sq")
            ssq = work.tile([P, 1], F32, tag="ssq")
            nc.scalar.activation(sq[:], src_psum, mybir.ActivationFunctionType.Square,
                                 accum_out=ssq[:])
            rms = work.tile([P, 1], F32, tag="rms")
            nc.scalar.activation(rms[:], ssq[:], mybir.ActivationFunctionType.Sqrt,
                                 bias=eps_col[:], scale=1.0 / H)
            rinv = work.tile([P, 1], F32, tag="rinv")
            nc.vector.reciprocal(rinv[:], rms[:])
            xn = work.tile([P, H], F32, tag="xn")
            nc.vector.scalar_tensor_tensor(
                xn[:], src_psum, rinv[:], scale_sb[:],
                mybir.AluOpType.mult, mybir.AluOpType.mult)
            # rope
            co = cos_all[:, si * half:(si + 1) * half]
            sn = sin_all[:, si * half:(si + 1) * half]
            x1 = xn[:, 0:half]
            x2 = xn[:, half:H]
            t1 = work.tile([P, half], F32, tag="t1")
            t2 = work.tile([P, half], F32, tag="t2")
            xr = work.tile([P, H], F32, tag="xr")
            nc.vector.tensor_mul(t1[:], x1, co)
            nc.vector.tensor_mul(t2[:], x2, sn)
            nc.vector.tensor_sub(xr[:, 0:half], t1[:], t2[:])
            nc.vector.tensor_mul(t1[:], x2, co)
            nc.vector.tensor_mul(t2[:], x1, sn)
            nc.vector.tensor_add(xr[:, half:H], t1[:], t2[:])
            # transpose -> dst
            pt = psum_t.tile([P, P], F32, tag="pt")
            nc.tensor.transpose(pt[:], xr[:], ident[:])
            nc.vector.tensor_copy(dst_sb, pt[:])

        # ---- Phase A: K/V projection, norm+rope K, build kT and v ----
        with tc.tile_pool(name=pfx + "pa_x", bufs=3) as xp, \
             tc.tile_pool(name=pfx + "pa_ps", bufs=1, space="PSUM") as pskv, \
             tc.tile_pool(name=pfx + "pa_pt", bufs=2, space="PSUM") as pst, \
             tc.tile_pool(name=pfx + "pa_w", bufs=2) as work:
            for sc in range(S_CHUNKS):
                xt = [None] * D_TILES
                ps = [pskv.tile([P, 2 * H], F32, tag=f"kv{j}", name=f"pskv{j}") for j in range(4)]
                for d in range(D_TILES):
                    xt[d] = xp.tile([P, FREE], MM_DT, tag="xstage", name="xstage")
                    nc.sync.dma_start(
                        xt[d][:], xT[d * P:(d + 1) * P,
                                     sc * FREE:(sc + 1) * FREE])
                    for j in range(4):
                        nc.tensor.matmul(
                            ps[j][:], xt[d][:, j * P:(j + 1) * P],
                            wkv_sb[d][:], start=(d == 0), stop=(d == D_TILES - 1))
                for j in range(4):
                    si = sc * 4 + j
                    nc.vector.tensor_copy(v_sb[:, si * P:(si + 1) * P],
                                          ps[j][:, H:2 * H])
                    norm_rope_transpose(ps[j][:, 0:H], ksc_sb, si,
                                        kT_sb[:, si * P:(si + 1) * P],
                                        work, pst)

        # ---- Phase B: Q projection, norm+rope, build qT (2 heads) ----
        with tc.tile_pool(name=pfx + "pb_x", bufs=3) as xp, \
             tc.tile_pool(name=pfx + "pb_ps", bufs=1, space="PSUM") as psq, \
             tc.tile_pool(name=pfx + "pb_pt", bufs=2, space="PSUM") as pst, \
             tc.tile_pool(name=pfx + "pb_w", bufs=2) as work:
            for tch in range(T_CHUNKS):
                xt = [None] * D_TILES
                ps = [psq.tile([P, HEADS_PER_CORE * H], F32, tag=f"q{j}", name=f"psq{j}")
                      for j in range(4)]
                for d in range(D_TILES):
                    xt[d] = xp.tile([P, FREE], MM_DT, tag="xstage", name="xstage")
                    nc.sync.dma_start(
                        xt[d][:], xT[d * P:(d + 1) * P,
                                     T_CTX + tch * FREE:T_CTX + (tch + 1) * FREE])
                    for j in range(4):
                        nc.tensor.matmul(
                            ps[j][:], xt[d][:, j * P:(j + 1) * P],
                            wq_sb[d][:], start=(d == 0), stop=(d == D_TILES - 1))
                for j in range(4):
                    ti = tch * 4 + j
                    for hh in range(HEADS_PER_CORE):
                        norm_rope_transpose(
                            ps[j][:, hh * H:(hh + 1) * H], qsc_sb,
                            NOISE_TILE0 + ti,
                            qT_sb[:, hh * T_NOISE + ti * P:
                                  hh * T_NOISE + (ti + 1) * P],
                            work, pst)

        # ---- Phase C: attention ----
        PAIR = 2 * FREE   # exp processes two score banks at once
        with tc.tile_pool(name=pfx + "pc_sc", bufs=2, space="PSUM") as psc, \
             tc.tile_pool(name=pfx + "pc_av", bufs=2, space="PSUM") as pav, \
             tc.tile_pool(name=pfx + "pc_r", bufs=2, space="PSUM") as pr, \
             tc.tile_pool(name=pfx + "pc_exp", bufs=3) as pexp:
            for hh in range(HEADS_PER_CORE):
                for tch in range(T_CHUNKS):
                    qslice = qT_sb[:, hh * T_NOISE + tch * FREE:
                                   hh * T_NOISE + (tch + 1) * FREE]
                    av = pav.tile([P, FREE], F32, tag="av")
                    rr = pr.tile([1, FREE], F32, tag="rr")
                    for sp in range(S_TILES // 2):
                        sc_ps = psc.tile([P, PAIR], F32, tag="sc")
                        ex = pexp.tile([P, PAIR], MM_DT, tag="ex")
                        for u in range(2):
                            si = sp * 2 + u
                            nc.tensor.matmul(
                                sc_ps[:, u * FREE:(u + 1) * FREE],
                                kT_sb[:, si * P:(si + 1) * P], qslice,
                                start=True, stop=True)
                        nc.scalar.activation(ex[:], sc_ps[:],
                                             mybir.ActivationFunctionType.Exp,
                                             scale=INV_SQRT_H)
                        for u in range(2):
                            si = sp * 2 + u
                            first = si == 0
                            last = si == S_TILES - 1
                            nc.tensor.matmul(
                                av[:], v_sb[:, si * P:(si + 1) * P],
                                ex[:, u * FREE:(u + 1) * FREE],
                                start=first, stop=last)
                            nc.tensor.matmul(
                                rr[:], ones[:],
                                ex[:, u * FREE:(u + 1) * FREE],
                                start=first, stop=last)
                    nc.vector.tensor_copy(
                        oT_sb[:, hh * T_NOISE + tch * FREE:
                              hh * T_NOISE + (tch + 1) * FREE], av[:])
                    nc.vector.reciprocal(
                        r_all[0:1, hh * T_NOISE + tch * FREE:
                              hh * T_NOISE + (tch + 1) * FREE], rr[:])

        # recip row -> per-partition columns (SBUF->SBUF DMA transpose, tiny)
        for hh in range(HEADS_PER_CORE):
            for ti in range(T_TILES):
                nc.sync.dma_start(
                    rcol[:, hh * T_TILES + ti:hh * T_TILES + ti + 1],
                    r_all[0:1, hh * T_NOISE + ti * P:hh * T_NOISE + (ti + 1) * P])

        # ---- Phase D: o-projection + deferred softmax normalization ----
        with tc.tile_pool(name=pfx + "pd_ps", bufs=2, space="PSUM") as pso, \
             tc.tile_pool(name=pfx + "pd_w", bufs=3) as work:
            for ti in range(T_TILES):
                for dc in range(D_CHUNKS):
                    po = [pso.tile([P, FREE], F32, tag=f"po{h}", name=f"po{h}")
                          for h in range(HEADS_PER_CORE)]
                    for h in range(HEADS_PER_CORE):
                        nc.tensor.matmul(
                            po[h][:],
                            oT_sb[:, h * T_NOISE + ti * P:h * T_NOISE + (ti + 1) * P],
                            wo_sb[h][:, dc * FREE:(dc + 1) * FREE],
                            start=True, stop=True)
                    tmp = work.tile([P, FREE], F32, tag="tmp")
                    nc.vector.tensor_scalar_mul(
                        tmp[:], po[1][:],
                        rcol[:, T_TILES + ti:T_TILES + ti + 1])
                    ot = work.tile([P, FREE], F32, tag="ot")
                    nc.vector.scalar_tensor_tensor(
                        ot[:], po[0][:], rcol[:, ti:ti + 1], tmp[:],
                        mybir.AluOpType.mult, mybir.AluOpType.add)
                    nc.sync.dma_start(
                        out[ti * P:(ti + 1) * P, dc * FREE:(dc + 1) * FREE],
                        ot[:])


def _get_program(reps=1):
    key = f"prog{reps}"
    if key not in _CACHE:
        _CACHE[key] = _build_program(reps)
    return _CACHE[key]


def prepare_in_maps(x_noise, target_hidden, Wq, Wk, Wv, Wo, q_scale, k_scale,
                    noise_positions, ctx_positions):
    x_noise = np.asarray(x_noise, dtype=np.float32)
    target_hidden = np.asarray(target_hidden, dtype=np.float32)
    Wq = np.asarray(Wq, dtype=np.float32)
    Wk = np.asarray(Wk, dtype=np.float32)
    Wv = np.asarray(Wv, dtype=np.float32)
    Wo = np.asarray(Wo, dtype=np.float32)
    q_scale = np.asarray(q_scale, dtype=np.float32)
    k_scale = np.asarray(k_scale, dtype=np.float32)

    x_all = np.concatenate([target_hidden, x_noise], axis=0)       # (S, D)
    xT = np.ascontiguousarray(x_all.T)                             # (D, S)
    pos_all = np.concatenate(
        [np.asarray(ctx_positions), np.asarray(noise_positions)]
    ).astype(np.float32)
    posr = np.ascontiguousarray(pos_all.reshape(S_TILES, P, 1))
    half = H // 2
    inv_freq = (ROPE_THETA ** (-np.arange(half, dtype=np.float32) * 2.0 / H)
                ).astype(np.float32)
    invfb = np.ascontiguousarray(np.broadcast_to(inv_freq, (P, half)))
    qscaleb = np.ascontiguousarray(np.broadcast_to(q_scale, (P, H)))
    kscaleb = np.ascontiguousarray(np.broadcast_to(k_scale, (P, H)))

    in_maps = []
    for c in range(N_CORES):
        wkv = np.ascontiguousarray(
            np.concatenate([Wk[:, c, :], Wv[:, c, :]], axis=1))     # (D, 256)
        wq = np.ascontiguousarray(
            Wq[:, c * HEADS_PER_CORE:(c + 1) * HEADS_PER_CORE, :]
            .reshape(D, HEADS_PER_CORE * H))                        # (D, 256)
        wo = np.ascontiguousarray(
            Wo[c * HEADS_PER_CORE:(c + 1) * HEADS_PER_CORE])        # (2,128,D)
        in_maps.append({
            "xT": xT, "wkv": wkv, "wq": wq, "wo": wo,
            "posr": posr, "invfb": invfb,
            "qscaleb": qscaleb, "kscaleb": kscaleb,
            "onesb": np.ones((P, 1), dtype=np.float32),
        })
    return in_maps


def kernel(**inputs):
    in_maps = prepare_in_maps(**inputs)
    nc, out_name = _get_program()
    res = run_bass_kernel_spmd(nc, in_maps, core_ids=list(range(N_CORES)))
    acc = np.zeros((T_NOISE, D), dtype=np.float32)
    for r in res.results:
        acc += r[out_name]
    return acc


def run_traced(inputs, **kw):
    """Run once with NTFF tracing; returns BassKernelResults (exec_time_ns)."""
    in_maps = prepare_in_maps(**inputs)
    nc, out_name = _get_program()
    return run_bass_kernel_spmd(nc, in_maps, core_ids=list(range(N_CORES)),
                                trace=True, **kw)



# revision 10
# speedup vs baseline: 1.9821x; 1.9821x over previous
"""DFlashAttention Trainium2 kernel (8-core tensor-parallel over attention heads).

Shapes (hardcoded): D=2048, N=16 q-heads, K=8 kv-heads, H=128,
T_NOISE=2048 (query tokens), T_CTX=4096, S=6144 (kv tokens).

Sharding: core c owns q-heads {2c, 2c+1} and kv-head c (GQA groups=2).
Each core computes a partial (T, D) output (its 2 heads' slice of the
o-projection contraction); the host sums the 8 partials (TP unshard).

v2 design (all matmul operands fp16):
  - Phase A (merged QKV proj): x streamed once as fp16 in [128,16,1024]
    chunks (one DMA per chunk).  ctx chunks compute k|v (256-wide moving),
    noise chunks compute k|v|q0|q1 in one 512-wide moving matmul.
    RMS-norm + RoPE in token-partition layout, PE transpose -> kT/qT [h,s].
  - Phase C (attention): two head-streams software-pipelined per t-chunk:
    PE issue order per step p: scores(p) for both streams, then AV(p-1),
    so the PE never sits behind ACT's exp.  exp = e^(score/sqrt(H) - 6.6)
    -> fp16 (bias keeps probs in fp16 range; it cancels in normalization).
    Softmax denominators: fp16 tree-fold of prob tiles on DVE (2x mode)
    + one ones-matmul on the folded tile (kills the per-s-tile rowsum
    matmul chain of v1).  Normalization fused into the av->oT copy via a
    rank-1 broadcast matmul of 1/r.
  - Phase D (o-proj): per t-chunk, both heads accumulate into one PSUM
    bank group; DMA straight PSUM->HBM.  No vector-engine work.
"""

import sys

for _p in ("/opt/trn_rl_repo", "/root/.axon_site/_ro/trn_rl_repo"):
    if _p not in sys.path:
        sys.path.append(_p)

import math
import numpy as np

import concourse.bass as bass
import concourse.tile as tile
from concourse import bacc
from concourse import mybir
from concourse.bass_utils import run_bass_kernel_spmd
from concourse.masks import make_identity

D = 2048
N_HEADS = 16
K_HEADS = 8
H = 128
T_NOISE = 2048
T_CTX = 4096
S_ALL = T_CTX + T_NOISE          # 6144
EPS = 1e-6
ROPE_THETA = 1e6
N_CORES = 8
HEADS_PER_CORE = N_HEADS // N_CORES   # 2

P = 128                       # partition dim
HALF = H // 2                 # 64
S_TILES = S_ALL // P          # 48
T_TILES = T_NOISE // P        # 16
NOISE_TILE0 = T_CTX // P      # 32  (noise tokens are s-tiles 32..47)
D_TILES = D // P              # 16
FREE = 512
PAIR = 2 * FREE               # 1024
CHUNK = 1024                  # tokens per x DMA chunk
N_CHUNKS = S_ALL // CHUNK     # 6 (chunks 4,5 are the noise tokens)
T_CHUNKS = T_NOISE // FREE    # 4
SP_PAIRS = S_TILES // 2       # 24 score pairs per (head, t-chunk)

F32 = mybir.dt.float32
F16 = mybir.dt.float16

TWO_PI = 2.0 * math.pi
INV_SQRT_H = 1.0 / math.sqrt(H)
EXP_BIAS = -6.6               # e^(13.69-6.6)*48*1.025 < 65504 (fp16 safe)

_CACHE = {}


def _build_program(reps=1):
    nc = bacc.Bacc("TRN2", target_bir_lowering=False, debug=False,
                   num_devices=N_CORES)

    # xs[p, d, s] = x_all[s, d*128+p]  (host pre-swizzled)
    xs = nc.dram_tensor("xs", [P, D_TILES, S_ALL], F16,
                        kind="ExternalInput").ap()
    # wkvq[p, d, :] = [Wk | Wv | Wq0 | Wq1][d*128+p, :]
    wkvq = nc.dram_tensor("wkvq", [P, D_TILES, 4 * H], F16,
                          kind="ExternalInput").ap()
    # wob[p, j, :] = Wo[head j][p, :]
    wob = nc.dram_tensor("wob", [P, HEADS_PER_CORE, D], F16,
                         kind="ExternalInput").ap()
    # post[p, j] = position of token j*128+p
    post = nc.dram_tensor("post", [P, S_TILES], F32,
                          kind="ExternalInput").ap()
    invfb = nc.dram_tensor("invfb", [P, HALF], F32,
                           kind="ExternalInput").ap()
    qscaleb = nc.dram_tensor("qscaleb", [P, H], F32,
                             kind="ExternalInput").ap()
    kscaleb = nc.dram_tensor("kscaleb", [P, H], F32,
                             kind="ExternalInput").ap()
    out = nc.dram_tensor("out", [T_NOISE, D], F16, kind="ExternalOutput").ap()

    with tile.TileContext(nc) as tc:
        for rep in range(reps):
            _emit(nc, tc, xs, wkvq, wob, post, invfb, qscaleb, kscaleb,
                  out, pfx=f"r{rep}_")
    nc.compile()
    return nc, "out"


def _emit(nc, tc, xs, wkvq, wob, post, invfb, qscaleb, kscaleb, out, pfx=""):
    import contextlib
    ctx = contextlib.ExitStack()
    with ctx:
        const = ctx.enter_context(tc.tile_pool(name=pfx + "const", bufs=1))
        persist = ctx.enter_context(tc.tile_pool(name=pfx + "persist", bufs=1))

        # ---- constants ----
        ident = const.tile([P, P], F16, tag="ident")
        make_identity(nc, ident[:])
        ones16 = const.tile([P, 1], F16, tag="ones16")
        nc.vector.memset(ones16[:], 1.0)
        ones_row = const.tile([1, P], F32, tag="ones_row")
        nc.vector.memset(ones_row[:], 1.0)
        invf_sb = const.tile([P, HALF], F32, tag="invf")
        nc.sync.dma_start(invf_sb[:], invfb[:])
        qsc_sb = const.tile([P, H], F32, tag="qsc")
        nc.sync.dma_start(qsc_sb[:], qscaleb[:])
        ksc_sb = const.tile([P, H], F32, tag="ksc")
        nc.sync.dma_start(ksc_sb[:], kscaleb[:])
        pos_sb = const.tile([P, S_TILES], F32, tag="pos")
        nc.sync.dma_start(pos_sb[:], post[:])
        eps_col = const.tile([P, 1], F32, tag="eps")
        nc.vector.memset(eps_col[:], EPS)
        ebias_col = const.tile([P, 1], F32, tag="ebias")
        nc.vector.memset(ebias_col[:], EXP_BIAS)
        wkvq_sb = const.tile([P, D_TILES * 4 * H], F16, tag="wkvq")
        nc.sync.dma_start(wkvq_sb[:], wkvq[:])
        wo_sb = const.tile([P, HEADS_PER_CORE * D], F16, tag="wo")
        nc.sync.dma_start(wo_sb[:], wob[:])

        # ---- persistent activations ----
        sin_all = persist.tile([P, S_TILES * HALF], F16, tag="sin")
        cos_all = persist.tile([P, S_TILES * HALF], F16, tag="cos")
        kT_sb = persist.tile([P, S_ALL], F16, tag="kT")
        v_sb = persist.tile([P, S_ALL], F16, tag="v")     # [s-tile, h] blocks
        qT_sb = persist.tile([P, HEADS_PER_CORE * T_NOISE], F16, tag="qT")
        oT_sb = persist.tile([P, HEADS_PER_CORE * T_NOISE], F16, tag="oT")

        # ---- RoPE sin/cos tables for all 48 token tiles ----
        CW1, CW2, CW3 = 6.28125, 0.0019353071693331003, 1.0253131677018246e-11
        HGRP = S_TILES // 2
        HW_ = HGRP * HALF
        with tc.tile_pool(name=pfx + "ropebuild", bufs=1) as rp:
            for g in range(2):
                ang = rp.tile([P, HW_], F32, tag="ang", name="ang")
                kq = rp.tile([P, HW_], F32, tag="kq", name="kq")
                ki = rp.tile([P, HW_], mybir.dt.int32, tag="ki", name="ki")
                wrap = rp.tile([P, HW_], F32, tag="wrap", name="wrap")
                for j in range(HGRP):
                    si = g * HGRP + j
                    nc.vector.tensor_scalar_mul(
                        ang[:, j * HALF:(j + 1) * HALF], invf_sb[:, :],
                        pos_sb[:, si:si + 1])
                nc.vector.tensor_scalar_mul(kq[:], ang[:], 1.0 / TWO_PI)
                nc.vector.tensor_copy(ki[:], kq[:])
                nc.vector.tensor_copy(kq[:], ki[:])
                nc.vector.cody_waite_cascade(ang[:], ang[:], kq[:],
                                             CW1, CW2, CW3)
                dst = slice(g * HW_, (g + 1) * HW_)
                nc.vector.add_range_wrap(wrap[:], ang[:], 0.0, math.pi, TWO_PI)
                nc.scalar.activation(sin_all[:, dst], wrap[:],
                                     mybir.ActivationFunctionType.Sin)
                nc.vector.add_range_wrap(wrap[:], ang[:], math.pi / 2, math.pi,
                                         TWO_PI)
                nc.scalar.activation(cos_all[:, dst], wrap[:],
                                     mybir.ActivationFunctionType.Sin)

        def norm_rope_transpose(src_psum, scale_sb, si, dst_sb, work, psum_t):
            """src_psum [P(tok),H] f32 -> rms-norm*scale -> rope -> transpose
            -> dst_sb [P(h), 128 tok] fp16. si = token-tile for positions."""
            sq = work.tile([P, H], F32, tag="sq")
            ssq = work.tile([P, 1], F32, tag="ssq")
            nc.scalar.activation(sq[:], src_psum,
                                 mybir.ActivationFunctionType.Square,
                                 accum_out=ssq[:])
            rms = work.tile([P, 1], F32, tag="rms")
            nc.scalar.activation(rms[:], ssq[:],
                                 mybir.ActivationFunctionType.Sqrt,
                                 bias=eps_col[:], scale=1.0 / H)
            rinv = work.tile([P, 1], F32, tag="rinv")
            nc.vector.reciprocal(rinv[:], rms[:])
            xn = work.tile([P, H], F16, tag="xn")
            nc.vector.scalar_tensor_tensor(
                xn[:], src_psum, rinv[:], scale_sb[:],
                mybir.AluOpType.mult, mybir.AluOpType.mult)
            co = cos_all[:, si * HALF:(si + 1) * HALF]
            sn = sin_all[:, si * HALF:(si + 1) * HALF]
            x1 = xn[:, 0:HALF]
            x2 = xn[:, HALF:H]
            t1 = work.tile([P, HALF], F16, tag="t1")
            t2 = work.tile([P, HALF], F16, tag="t2")
            xr = work.tile([P, H], F16, tag="xr")
            nc.vector.tensor_mul(t1[:], x1, co)
            nc.vector.tensor_mul(t2[:], x2, sn)
            nc.vector.tensor_sub(xr[:, 0:HALF], t1[:], t2[:])
            nc.vector.tensor_mul(t1[:], x2, co)
            nc.vector.tensor_mul(t2[:], x1, sn)
            nc.vector.tensor_add(xr[:, HALF:H], t1[:], t2[:])
            pt = psum_t.tile([P, P], F16, tag="pt")
            nc.tensor.transpose(pt[:], xr[:], ident[:])
            nc.vector.tensor_copy(dst_sb, pt[:])

        # ---- Phase A: merged kvq projection -> kT, v, qT ----
        with tc.tile_pool(name=pfx + "pa_x", bufs=2) as xp, \
             tc.tile_pool(name=pfx + "pa_ps", bufs=1, space="PSUM") as pska, \
             tc.tile_pool(name=pfx + "pa_pt", bufs=2, space="PSUM") as pst, \
             tc.tile_pool(name=pfx + "pa_w", bufs=2) as work:
            for c in range(N_CHUNKS):
                noise = c >= 4
                W = 4 * H if noise else 2 * H
                xst = xp.tile([P, D_TILES, CHUNK], F16, tag="xst", name="xst")
                nc.sync.dma_start(xst[:], xs[:, :, c * CHUNK:(c + 1) * CHUNK])
                for half in range(2):
                    ps = [pska.tile([P, 4 * H], F32, tag=f"ps{t}",
                                    name=f"ps{t}") for t in range(4)]
                    for d in range(D_TILES):
                        for t in range(4):
                            tok = half * 4 + t
                            nc.tensor.matmul(
                                ps[t][:, 0:W],
                                xst[:, d, tok * P:(tok + 1) * P],
                                wkvq_sb[:, d * 4 * H:d * 4 * H + W],
                                start=(d == 0), stop=(d == D_TILES - 1))
                    for t in range(4):
                        si = c * 8 + half * 4 + t
                        nc.vector.tensor_copy(
                            v_sb[:, si * P:(si + 1) * P], ps[t][:, H:2 * H])
                        norm_rope_transpose(
                            ps[t][:, 0:H], ksc_sb, si,
                            kT_sb[:, si * P:(si + 1) * P], work, pst)
                        if noise:
                            ti = si - NOISE_TILE0
                            for hh in range(HEADS_PER_CORE):
                                norm_rope_transpose(
                                    ps[t][:, (2 + hh) * H:(3 + hh) * H],
                                    qsc_sb, si,
                                    qT_sb[:, hh * T_NOISE + ti * P:
                                          hh * T_NOISE + (ti + 1) * P],
                                    work, pst)

        # ---- Phase C + D: attention (2 head-streams) + o-projection ----
        with tc.tile_pool(name=pfx + "pc_sc", bufs=1, space="PSUM") as psc, \
             tc.tile_pool(name=pfx + "pc_sh", bufs=2, space="PSUM") as shared, \
             tc.tile_pool(name=pfx + "pc_rbc", bufs=1, space="PSUM") as prbc, \
             tc.tile_pool(name=pfx + "pc_ex", bufs=3) as pexp, \
             tc.tile_pool(name=pfx + "pc_rf", bufs=2) as prf, \
             tc.tile_pool(name=pfx + "pc_rv", bufs=2) as prv, \
             tc.tile_pool(name=pfx + "pc_po", bufs=4) as posb:
            sc = [psc.tile([P, PAIR], F32, tag=f"sc{st}", name=f"sc{st}")
                  for st in range(2)]
            for tch in range(T_CHUNKS):
                av = [shared.tile([P, FREE], F32, tag="av", name=f"av{st}")
                      for st in range(2)]
                rf = [prf.tile([P, PAIR], F16, tag=f"rf{st}", name=f"rf{st}")
                      for st in range(2)]
                qsl = [qT_sb[:, st * T_NOISE + tch * FREE:
                             st * T_NOISE + (tch + 1) * FREE]
                       for st in range(2)]
                prev_ex = [None, None]
                cur_ex = [None, None]
                for p in range(SP_PAIRS + 1):
                    if p < SP_PAIRS:
                        # scores for pair p, both streams
                        for st in range(2):
                            for u in range(2):
                                si = 2 * p + u
                                nc.tensor.matmul(
                                    sc[st][:, u * FREE:(u + 1) * FREE],
                                    kT_sb[:, si * P:(si + 1) * P], qsl[st],
                                    start=True, stop=True)
                        for st in range(2):
                            e = pexp.tile([P, PAIR], F16, tag=f"ex{st}",
                                          name=f"ex{st}")
                            nc.scalar.activation(
                                e[:], sc[st][:],
                                mybir.ActivationFunctionType.Exp,
                                bias=ebias_col[:], scale=INV_SQRT_H)
                            cur_ex[st] = e
                    if p >= 1:
                        q = p - 1
                        for st in range(2):
                            e = prev_ex[st]
                            nc.tensor.matmul(
                                av[st][:], v_sb[:, (2 * q) * P:(2 * q + 1) * P],
                                e[:, 0:FREE],
                                start=(q == 0), stop=False)
                            nc.tensor.matmul(
                                av[st][:],
                                v_sb[:, (2 * q + 1) * P:(2 * q + 2) * P],
                                e[:, FREE:PAIR],
                                start=False, stop=(q == SP_PAIRS - 1))
                        for st in range(2):
                            e = prev_ex[st]
                            if q == 0:
                                nc.vector.tensor_copy(rf[st][:], e[:])
                            else:
                                nc.vector.tensor_add(rf[st][:], rf[st][:], e[:])
                    prev_ex = list(cur_ex)

                # epilogue per stream: denominators + normalized oT
                for st in range(2):
                    rbc = prbc.tile([P, FREE], F32, tag="rbc", name="rbc")
                    nc.tensor.matmul(rbc[0:1, :], ones16[:], rf[st][:, 0:FREE],
                                     start=True, stop=False)
                    nc.tensor.matmul(rbc[0:1, :], ones16[:], rf[st][:, FREE:PAIR],
                                     start=False, stop=True)
                    rinv_r = prv.tile([1, FREE], F32, tag="rinv_r",
                                      name="rinv_r")
                    nc.vector.reciprocal(rinv_r[:], rbc[0:1, :])
                    nc.tensor.matmul(rbc[:, :], ones_row[:], rinv_r[:],
                                     start=True, stop=True)
                    rbs = prv.tile([P, FREE], F32, tag="rbs", name="rbs")
                    nc.vector.tensor_copy(rbs[:], rbc[:, :])
                    nc.vector.tensor_mul(
                        oT_sb[:, st * T_NOISE + tch * FREE:
                              st * T_NOISE + (tch + 1) * FREE],
                        av[st][:], rbs[:])

                # Phase D for this t-chunk: o-projection, both heads into
                # one accumulator, straight to HBM.
                for ti in range(4):
                    t0 = tch * FREE + ti * P
                    for dh in range(2):
                        po = [shared.tile([P, FREE], F32, tag="av",
                                          name=f"po{u}") for u in range(2)]
                        for st in range(2):
                            osl = oT_sb[:, st * T_NOISE + t0:
                                        st * T_NOISE + t0 + P]
                            for u in range(2):
                                nc.tensor.matmul(
                                    po[u][:], osl,
                                    wo_sb[:, st * D + dh * PAIR + u * FREE:
                                          st * D + dh * PAIR + (u + 1) * FREE],
                                    start=(st == 0), stop=(st == 1))
                        for u in range(2):
                            ob = posb.tile([P, FREE], F16, tag="ob",
                                           name="ob")
                            nc.vector.tensor_copy(ob[:], po[u][:])
                            nc.sync.dma_start(
                                out[t0:t0 + P,
                                    dh * PAIR + u * FREE:
                                    dh * PAIR + (u + 1) * FREE],
                                ob[:])


def _get_program(reps=1):
    key = f"prog{reps}"
    if key not in _CACHE:
        _CACHE[key] = _build_program(reps)
    return _CACHE[key]


def prepare_in_maps(x_noise, target_hidden, Wq, Wk, Wv, Wo, q_scale, k_scale,
                    noise_positions, ctx_positions):
    x_noise = np.asarray(x_noise, dtype=np.float32)
    target_hidden = np.asarray(target_hidden, dtype=np.float32)
    Wq = np.asarray(Wq, dtype=np.float32)
    Wk = np.asarray(Wk, dtype=np.float32)
    Wv = np.asarray(Wv, dtype=np.float32)
    Wo = np.asarray(Wo, dtype=np.float32)
    q_scale = np.asarray(q_scale, dtype=np.float32)
    k_scale = np.asarray(k_scale, dtype=np.float32)

    x_all = np.concatenate([target_hidden, x_noise], axis=0)       # (S, D)
    # xs[p, d, s] = x_all[s, d*128+p]
    xs = np.ascontiguousarray(
        x_all.T.reshape(D_TILES, P, S_ALL).transpose(1, 0, 2)
    ).astype(np.float16)
    pos_all = np.concatenate(
        [np.asarray(ctx_positions), np.asarray(noise_positions)]
    ).astype(np.float32)
    post = np.ascontiguousarray(pos_all.reshape(S_TILES, P).T)     # (P, 48)
    inv_freq = (ROPE_THETA ** (-np.arange(HALF, dtype=np.float32) * 2.0 / H)
                ).astype(np.float32)
    invfb = np.ascontiguousarray(np.broadcast_to(inv_freq, (P, HALF)))
    qscaleb = np.ascontiguousarray(np.broadcast_to(q_scale, (P, H)))
    kscaleb = np.ascontiguousarray(np.broadcast_to(k_scale, (P, H)))

    in_maps = []
    for c in range(N_CORES):
        wkvq = np.concatenate(
            [Wk[:, c, :], Wv[:, c, :],
             Wq[:, 2 * c, :], Wq[:, 2 * c + 1, :]], axis=1)        # (D, 512)
        wkvq = np.ascontiguousarray(
            wkvq.reshape(D_TILES, P, 4 * H).transpose(1, 0, 2)
        ).astype(np.float16)                                        # (P,16,512)
        wob = np.ascontiguousarray(
            Wo[2 * c:2 * c + 2].transpose(1, 0, 2)
        ).astype(np.float16)                                        # (P,2,D)
        in_maps.append({
            "xs": xs, "wkvq": wkvq, "wob": wob,
            "post": post, "invfb": invfb,
            "qscaleb": qscaleb, "kscaleb": kscaleb,
        })
    return in_maps


def kernel(**inputs):
    in_maps = prepare_in_maps(**inputs)
    nc, out_name = _get_program()
    res = run_bass_kernel_spmd(nc, in_maps, core_ids=list(range(N_CORES)))
    acc = np.zeros((T_NOISE, D), dtype=np.float32)
    for r in res.results:
        acc += r[out_name].astype(np.float32)
    return acc


def run_traced(inputs, **kw):
    """Run once with NTFF tracing; returns BassKernelResults (exec_time_ns)."""
    in_maps = prepare_in_maps(**inputs)
    nc, out_name = _get_program()
    return run_bass_kernel_spmd(nc, in_maps, core_ids=list(range(N_CORES)),
                                trace=True, **kw)


# revision 13
# speedup vs baseline: 2.0891x; 1.0539x over previous
"""DFlashAttention Trainium2 kernel (8-core tensor-parallel over attention heads).

Shapes (hardcoded): D=2048, N=16 q-heads, K=8 kv-heads, H=128,
T_NOISE=2048 (query tokens), T_CTX=4096, S=6144 (kv tokens).

Sharding: core c owns q-heads {2c, 2c+1} and kv-head c (GQA groups=2).
Each core computes a partial (T, D) output (its 2 heads' slice of the
o-projection contraction); the host sums the 8 partials (TP unshard).

v2 design (all matmul operands fp16):
  - Phase A (merged QKV proj): x streamed once as fp16 in [128,16,1024]
    chunks (one DMA per chunk).  ctx chunks compute k|v (256-wide moving),
    noise chunks compute k|v|q0|q1 in one 512-wide moving matmul.
    RMS-norm + RoPE in token-partition layout, PE transpose -> kT/qT [h,s].
  - Phase C (attention): two head-streams software-pipelined per t-chunk:
    PE issue order per step p: scores(p) for both streams, then AV(p-1),
    so the PE never sits behind ACT's exp.  exp = e^(score/sqrt(H) - 6.6)
    -> fp16 (bias keeps probs in fp16 range; it cancels in normalization).
    Softmax denominators: fp16 tree-fold of prob tiles on DVE (2x mode)
    + one ones-matmul on the folded tile (kills the per-s-tile rowsum
    matmul chain of v1).  Normalization fused into the av->oT copy via a
    rank-1 broadcast matmul of 1/r.
  - Phase D (o-proj): per t-chunk, both heads accumulate into one PSUM
    bank group; DMA straight PSUM->HBM.  No vector-engine work.
"""

import sys

for _p in ("/opt/trn_rl_repo", "/root/.axon_site/_ro/trn_rl_repo"):
    if _p not in sys.path:
        sys.path.append(_p)

import math
import numpy as np

import concourse.bass as bass
import concourse.tile as tile
from concourse import bacc
from concourse import mybir
from concourse.bass_utils import run_bass_kernel_spmd
from concourse.masks import make_identity

D = 2048
N_HEADS = 16
K_HEADS = 8
H = 128
T_NOISE = 2048
T_CTX = 4096
S_ALL = T_CTX + T_NOISE          # 6144
EPS = 1e-6
ROPE_THETA = 1e6
N_CORES = 8
HEADS_PER_CORE = N_HEADS // N_CORES   # 2

P = 128                       # partition dim
HALF = H // 2                 # 64
S_TILES = S_ALL // P          # 48
T_TILES = T_NOISE // P        # 16
NOISE_TILE0 = T_CTX // P      # 32  (noise tokens are s-tiles 32..47)
D_TILES = D // P              # 16
FREE = 512
PAIR = 2 * FREE               # 1024
CHUNK = 1024                  # tokens per x DMA chunk
N_CHUNKS = S_ALL // CHUNK     # 6 (chunks 4,5 are the noise tokens)
T_CHUNKS = T_NOISE // FREE    # 4
SP_PAIRS = S_TILES // 2       # 24 score pairs per (head, t-chunk)

F32 = mybir.dt.float32
F16 = mybir.dt.float16

TWO_PI = 2.0 * math.pi
INV_SQRT_H = 1.0 / math.sqrt(H)
EXP_BIAS = -6.6               # e^(13.69-6.6)*48*1.025 < 65504 (fp16 safe)

_CACHE = {}


def _build_program(reps=1):
    nc = bacc.Bacc("TRN2", target_bir_lowering=False, debug=False,
                   num_devices=N_CORES)

    # xs[p, d, s] = x_all[s, d*128+p]  (host pre-swizzled)
    xs = nc.dram_tensor("xs", [P, D_TILES, S_ALL], F16,
                        kind="ExternalInput").ap()
    # wkvq[p, d, :] = [Wk | Wv | Wq0 | Wq1][d*128+p, :]
    wkvq = nc.dram_tensor("wkvq", [P, D_TILES, 4 * H], F16,
                          kind="ExternalInput").ap()
    # wob[p, j, :] = Wo[head j][p, :]
    wob = nc.dram_tensor("wob", [P, HEADS_PER_CORE, D], F16,
                         kind="ExternalInput").ap()
    # post[p, j] = position of token j*128+p
    post = nc.dram_tensor("post", [P, S_TILES], F32,
                          kind="ExternalInput").ap()
    invfb = nc.dram_tensor("invfb", [P, HALF], F32,
                           kind="ExternalInput").ap()
    qscaleb = nc.dram_tensor("qscaleb", [P, H], F32,
                             kind="ExternalInput").ap()
    kscaleb = nc.dram_tensor("kscaleb", [P, H], F32,
                             kind="ExternalInput").ap()
    out = nc.dram_tensor("out", [T_NOISE, D], F16, kind="ExternalOutput").ap()

    with tile.TileContext(nc) as tc:
        for rep in range(reps):
            _emit(nc, tc, xs, wkvq, wob, post, invfb, qscaleb, kscaleb,
                  out, pfx=f"r{rep}_")
    nc.compile()
    return nc, "out"


def _emit(nc, tc, xs, wkvq, wob, post, invfb, qscaleb, kscaleb, out, pfx=""):
    import contextlib
    ctx = contextlib.ExitStack()
    with ctx:
        const = ctx.enter_context(tc.tile_pool(name=pfx + "const", bufs=1))
        persist = ctx.enter_context(tc.tile_pool(name=pfx + "persist", bufs=1))

        # ---- constants ----
        ident = const.tile([P, P], F16, tag="ident")
        make_identity(nc, ident[:])
        ones16 = const.tile([P, 1], F16, tag="ones16")
        nc.vector.memset(ones16[:], 1.0)
        ones_row = const.tile([1, P], F32, tag="ones_row")
        nc.vector.memset(ones_row[:], 1.0)
        invf_sb = const.tile([P, HALF], F32, tag="invf")
        nc.sync.dma_start(invf_sb[:], invfb[:])
        qsc_sb = const.tile([P, H], F32, tag="qsc")
        nc.sync.dma_start(qsc_sb[:], qscaleb[:])
        ksc_sb = const.tile([P, H], F32, tag="ksc")
        nc.sync.dma_start(ksc_sb[:], kscaleb[:])
        pos_sb = const.tile([P, S_TILES], F32, tag="pos")
        nc.sync.dma_start(pos_sb[:], post[:])
        eps_col = const.tile([P, 1], F32, tag="eps")
        nc.vector.memset(eps_col[:], EPS)
        ebias_col = const.tile([P, 1], F32, tag="ebias")
        nc.vector.memset(ebias_col[:], EXP_BIAS)
        wkvq_sb = const.tile([P, D_TILES * 4 * H], F16, tag="wkvq")
        nc.sync.dma_start(wkvq_sb[:], wkvq[:])
        wo_sb = const.tile([P, HEADS_PER_CORE * D], F16, tag="wo")
        nc.sync.dma_start(wo_sb[:], wob[:])

        # ---- persistent activations ----
        sin_all = persist.tile([P, S_TILES * HALF], F16, tag="sin")
        cos_all = persist.tile([P, S_TILES * HALF], F16, tag="cos")
        kT_sb = persist.tile([P, S_ALL], F16, tag="kT")
        v_sb = persist.tile([P, S_ALL], F16, tag="v")     # [s-tile, h] blocks
        qT_sb = persist.tile([P, HEADS_PER_CORE * T_NOISE], F16, tag="qT")
        oT_sb = persist.tile([P, HEADS_PER_CORE * T_NOISE], F16, tag="oT")

        # ---- RoPE sin/cos tables for all 48 token tiles ----
        CW1, CW2, CW3 = 6.28125, 0.0019353071693331003, 1.0253131677018246e-11
        HGRP = S_TILES // 2
        HW_ = HGRP * HALF
        with tc.tile_pool(name=pfx + "ropebuild", bufs=1) as rp:
            for g in range(2):
                ang = rp.tile([P, HW_], F32, tag="ang", name="ang")
                kq = rp.tile([P, HW_], F32, tag="kq", name="kq")
                ki = rp.tile([P, HW_], mybir.dt.int32, tag="ki", name="ki")
                wrap = rp.tile([P, HW_], F32, tag="wrap", name="wrap")
                for j in range(HGRP):
                    si = g * HGRP + j
                    nc.vector.tensor_scalar_mul(
                        ang[:, j * HALF:(j + 1) * HALF], invf_sb[:, :],
                        pos_sb[:, si:si + 1])
                nc.vector.tensor_scalar_mul(kq[:], ang[:], 1.0 / TWO_PI)
                nc.vector.tensor_copy(ki[:], kq[:])
                nc.vector.tensor_copy(kq[:], ki[:])
                nc.vector.cody_waite_cascade(ang[:], ang[:], kq[:],
                                             CW1, CW2, CW3)
                dst = slice(g * HW_, (g + 1) * HW_)
                nc.vector.add_range_wrap(wrap[:], ang[:], 0.0, math.pi, TWO_PI)
                nc.scalar.activation(sin_all[:, dst], wrap[:],
                                     mybir.ActivationFunctionType.Sin)
                nc.vector.add_range_wrap(wrap[:], ang[:], math.pi / 2, math.pi,
                                         TWO_PI)
                nc.scalar.activation(cos_all[:, dst], wrap[:],
                                     mybir.ActivationFunctionType.Sin)

        def norm_rope_transpose(src_psum, scale_sb, si, dst_sb, work, psum_t):
            """src_psum [P(tok),H] f32 -> rms-norm*scale -> rope -> transpose
            -> dst_sb [P(h), 128 tok] fp16. si = token-tile for positions."""
            sq = work.tile([P, H], F32, tag="sq")
            ssq = work.tile([P, 1], F32, tag="ssq")
            nc.scalar.activation(sq[:], src_psum,
                                 mybir.ActivationFunctionType.Square,
                                 accum_out=ssq[:])
            rms = work.tile([P, 1], F32, tag="rms")
            nc.scalar.activation(rms[:], ssq[:],
                                 mybir.ActivationFunctionType.Sqrt,
                                 bias=eps_col[:], scale=1.0 / H)
            rinv = work.tile([P, 1], F32, tag="rinv")
            nc.vector.reciprocal(rinv[:], rms[:])
            xn = work.tile([P, H], F16, tag="xn")
            nc.vector.scalar_tensor_tensor(
                xn[:], src_psum, rinv[:], scale_sb[:],
                mybir.AluOpType.mult, mybir.AluOpType.mult)
            co = cos_all[:, si * HALF:(si + 1) * HALF]
            sn = sin_all[:, si * HALF:(si + 1) * HALF]
            x1 = xn[:, 0:HALF]
            x2 = xn[:, HALF:H]
            t1 = work.tile([P, HALF], F16, tag="t1")
            t2 = work.tile([P, HALF], F16, tag="t2")
            xr = work.tile([P, H], F16, tag="xr")
            nc.vector.tensor_mul(t1[:], x1, co)
            nc.vector.tensor_mul(t2[:], x2, sn)
            nc.vector.tensor_sub(xr[:, 0:HALF], t1[:], t2[:])
            nc.vector.tensor_mul(t1[:], x2, co)
            nc.vector.tensor_mul(t2[:], x1, sn)
            nc.vector.tensor_add(xr[:, HALF:H], t1[:], t2[:])
            pt = psum_t.tile([P, P], F16, tag="pt")
            nc.tensor.transpose(pt[:], xr[:], ident[:])
            nc.scalar.copy(dst_sb, pt[:])

        # ---- Phase A: merged kvq projection -> kT, v, qT ----
        with tc.tile_pool(name=pfx + "pa_x", bufs=2) as xp, \
             tc.tile_pool(name=pfx + "pa_ps", bufs=2, space="PSUM") as pska, \
             tc.tile_pool(name=pfx + "pa_pt", bufs=2, space="PSUM") as pst, \
             tc.tile_pool(name=pfx + "pa_w", bufs=2) as work:
            for c in range(N_CHUNKS):
                noise = c >= 4
                W = 4 * H if noise else 2 * H
                # x chunk staged in two halves so the first matmuls can
                # start before the whole chunk has landed
                xst = [xp.tile([P, D_TILES // 2, CHUNK], F16, tag=f"xst{hf}",
                               name=f"xst{hf}") for hf in range(2)]
                for hf in range(2):
                    nc.sync.dma_start(
                        xst[hf][:],
                        xs[:, hf * 8:(hf + 1) * 8, c * CHUNK:(c + 1) * CHUNK])
                # 8 token-tiles per chunk, processed in pairs (2-deep PSUM)
                for pairi in range(4):
                    ps = [pska.tile([P, 4 * H], F32, tag=f"ps{u}",
                                    name=f"ps{u}") for u in range(2)]
                    for d in range(D_TILES):
                        for u in range(2):
                            tok = pairi * 2 + u
                            nc.tensor.matmul(
                                ps[u][:, 0:W],
                                xst[d // 8][:, d % 8, tok * P:(tok + 1) * P],
                                wkvq_sb[:, d * 4 * H:d * 4 * H + W],
                                start=(d == 0), stop=(d == D_TILES - 1))
                    for u in range(2):
                        si = c * 8 + pairi * 2 + u
                        nc.vector.tensor_copy(
                            v_sb[:, si * P:(si + 1) * P], ps[u][:, H:2 * H])
                        norm_rope_transpose(
                            ps[u][:, 0:H], ksc_sb, si,
                            kT_sb[:, si * P:(si + 1) * P], work, pst)
                        if noise:
                            ti = si - NOISE_TILE0
                            for hh in range(HEADS_PER_CORE):
                                norm_rope_transpose(
                                    ps[u][:, (2 + hh) * H:(3 + hh) * H],
                                    qsc_sb, si,
                                    qT_sb[:, hh * T_NOISE + ti * P:
                                          hh * T_NOISE + (ti + 1) * P],
                                    work, pst)

        # ---- Phase C + D: attention (2 head-streams) + o-projection ----
        # PSUM: sc0,sc1 (2 banks each) + av0,av1 (1 each) + po/rbc pool (2)
        with tc.tile_pool(name=pfx + "pc_sc", bufs=1, space="PSUM") as psc, \
             tc.tile_pool(name=pfx + "pc_av", bufs=1, space="PSUM") as pav, \
             tc.tile_pool(name=pfx + "pc_po", bufs=2, space="PSUM") as ppo, \
             tc.tile_pool(name=pfx + "pc_ex", bufs=3) as pexp, \
             tc.tile_pool(name=pfx + "pc_rf", bufs=2) as prf, \
             tc.tile_pool(name=pfx + "pc_rv", bufs=2) as prv, \
             tc.tile_pool(name=pfx + "pc_ob", bufs=4) as posb:
            sc = [psc.tile([P, PAIR], F32, tag=f"sc{st}", name=f"sc{st}")
                  for st in range(2)]

            def emit_d_piece(dtch, ti, dh):
                """o-projection for t-tile ti, D-half dh of t-chunk dtch."""
                t0 = dtch * FREE + ti * P
                po = [ppo.tile([P, FREE], F32, tag="po", name=f"po{u}")
                      for u in range(2)]
                for st in range(2):
                    osl = oT_sb[:, st * T_NOISE + t0:st * T_NOISE + t0 + P]
                    for u in range(2):
                        nc.tensor.matmul(
                            po[u][:], osl,
                            wo_sb[:, st * D + dh * PAIR + u * FREE:
                                  st * D + dh * PAIR + (u + 1) * FREE],
                            start=(st == 0), stop=(st == 1))
                for u in range(2):
                    ob = posb.tile([P, FREE], F16, tag="ob", name="ob")
                    nc.vector.tensor_copy(ob[:], po[u][:])
                    nc.sync.dma_start(
                        out[t0:t0 + P,
                            dh * PAIR + u * FREE:dh * PAIR + (u + 1) * FREE],
                        ob[:])

            d_queue = []
            for tch in range(T_CHUNKS):
                av = [pav.tile([P, FREE], F32, tag=f"av{st}", name=f"av{st}")
                      for st in range(2)]
                # two denominator accumulators per stream: rfa on DVE,
                # rfb on the (otherwise idle) Pool engine
                rfa = [prf.tile([P, PAIR], F16, tag=f"rfa{st}",
                                name=f"rfa{st}") for st in range(2)]
                rfb = [prf.tile([P, PAIR], F16, tag=f"rfb{st}",
                                name=f"rfb{st}") for st in range(2)]
                qsl = [qT_sb[:, st * T_NOISE + tch * FREE:
                             st * T_NOISE + (tch + 1) * FREE]
                       for st in range(2)]
                prev_ex = [None, None]
                cur_ex = [None, None]
                na = [0, 0]
                nb = [0, 0]
                for p in range(SP_PAIRS + 1):
                    if p < SP_PAIRS:
                        # scores for pair p; stationary kT tile shared
                        # between the two streams (u-outer)
                        for u in range(2):
                            si = 2 * p + u
                            for st in range(2):
                                nc.tensor.matmul(
                                    sc[st][:, u * FREE:(u + 1) * FREE],
                                    kT_sb[:, si * P:(si + 1) * P], qsl[st],
                                    start=True, stop=True)
                        for st in range(2):
                            e = pexp.tile([P, PAIR], F16, tag=f"ex{st}",
                                          name=f"ex{st}")
                            nc.scalar.activation(
                                e[:], sc[st][:],
                                mybir.ActivationFunctionType.Exp,
                                bias=ebias_col[:], scale=INV_SQRT_H)
                            cur_ex[st] = e
                    if p >= 1:
                        q = p - 1
                        for u in range(2):
                            si = 2 * q + u
                            for st in range(2):
                                nc.tensor.matmul(
                                    av[st][:],
                                    v_sb[:, si * P:(si + 1) * P],
                                    prev_ex[st][:, u * FREE:(u + 1) * FREE],
                                    start=(q == 0 and u == 0),
                                    stop=(q == SP_PAIRS - 1 and u == 1))
                        for st in range(2):
                            e = prev_ex[st]
                            if q % 3 == 2:      # Pool engine accumulator
                                if nb[st] == 0:
                                    nc.gpsimd.tensor_copy(rfb[st][:], e[:])
                                else:
                                    nc.gpsimd.tensor_add(rfb[st][:],
                                                         rfb[st][:], e[:])
                                nb[st] += 1
                            else:               # DVE accumulator
                                if na[st] == 0:
                                    nc.vector.tensor_copy(rfa[st][:], e[:])
                                else:
                                    nc.vector.tensor_add(rfa[st][:],
                                                         rfa[st][:], e[:])
                                na[st] += 1
                        # interleave previous chunk's o-projection
                        if d_queue and 2 <= p <= 17 and p % 2 == 0:
                            emit_d_piece(*d_queue.pop(0))
                    prev_ex = list(cur_ex)

                # epilogue per stream: denominators + normalized oT
                for st in range(2):
                    rbc = ppo.tile([P, FREE], F32, tag="po", name="rbc")
                    nc.tensor.matmul(rbc[0:1, :], ones16[:],
                                     rfa[st][:, 0:FREE],
                                     start=True, stop=False)
                    nc.tensor.matmul(rbc[0:1, :], ones16[:],
                                     rfa[st][:, FREE:PAIR],
                                     start=False, stop=False)
                    nc.tensor.matmul(rbc[0:1, :], ones16[:],
                                     rfb[st][:, 0:FREE],
                                     start=False, stop=False)
                    nc.tensor.matmul(rbc[0:1, :], ones16[:],
                                     rfb[st][:, FREE:PAIR],
                                     start=False, stop=True)
                    rinv_r = prv.tile([1, FREE], F32, tag="rinv_r",
                                      name="rinv_r")
                    nc.vector.reciprocal(rinv_r[:], rbc[0:1, :])
                    nc.tensor.matmul(rbc[:, :], ones_row[:], rinv_r[:],
                                     start=True, stop=True)
                    rbs = prv.tile([P, FREE], F32, tag="rbs", name="rbs")
                    nc.vector.tensor_copy(rbs[:], rbc[:, :])
                    nc.vector.tensor_mul(
                        oT_sb[:, st * T_NOISE + tch * FREE:
                              st * T_NOISE + (tch + 1) * FREE],
                        av[st][:], rbs[:])
                d_queue.extend((tch, ti, dh)
                               for ti in range(4) for dh in range(2))
            for piece in d_queue:
                emit_d_piece(*piece)


def _get_program(reps=1):
    key = f"prog{reps}"
    if key not in _CACHE:
        _CACHE[key] = _build_program(reps)
    return _CACHE[key]


def prepare_in_maps(x_noise, target_hidden, Wq, Wk, Wv, Wo, q_scale, k_scale,
                    noise_positions, ctx_positions):
    x_noise = np.asarray(x_noise, dtype=np.float32)
    target_hidden = np.asarray(target_hidden, dtype=np.float32)
    Wq = np.asarray(Wq, dtype=np.float32)
    Wk = np.asarray(Wk, dtype=np.float32)
    Wv = np.asarray(Wv, dtype=np.float32)
    Wo = np.asarray(Wo, dtype=np.float32)
    q_scale = np.asarray(q_scale, dtype=np.float32)
    k_scale = np.asarray(k_scale, dtype=np.float32)

    x_all = np.concatenate([target_hidden, x_noise], axis=0)       # (S, D)
    # xs[p, d, s] = x_all[s, d*128+p]
    xs = np.ascontiguousarray(
        x_all.T.reshape(D_TILES, P, S_ALL).transpose(1, 0, 2)
    ).astype(np.float16)
    pos_all = np.concatenate(
        [np.asarray(ctx_positions), np.asarray(noise_positions)]
    ).astype(np.float32)
    post = np.ascontiguousarray(pos_all.reshape(S_TILES, P).T)     # (P, 48)
    inv_freq = (ROPE_THETA ** (-np.arange(HALF, dtype=np.float32) * 2.0 / H)
                ).astype(np.float32)
    invfb = np.ascontiguousarray(np.broadcast_to(inv_freq, (P, HALF)))
    qscaleb = np.ascontiguousarray(np.broadcast_to(q_scale, (P, H)))
    kscaleb = np.ascontiguousarray(np.broadcast_to(k_scale, (P, H)))

    in_maps = []
    for c in range(N_CORES):
        wkvq = np.concatenate(
            [Wk[:, c, :], Wv[:, c, :],
             Wq[:, 2 * c, :], Wq[:, 2 * c + 1, :]], axis=1)        # (D, 512)
        wkvq = np.ascontiguousarray(
            wkvq.reshape(D_TILES, P, 4 * H).transpose(1, 0, 2)
        ).astype(np.float16)                                        # (P,16,512)
        wob = np.ascontiguousarray(
            Wo[2 * c:2 * c + 2].transpose(1, 0, 2)
        ).astype(np.float16)                                        # (P,2,D)
        in_maps.append({
            "xs": xs, "wkvq": wkvq, "wob": wob,
            "post": post, "invfb": invfb,
            "qscaleb": qscaleb, "kscaleb": kscaleb,
        })
    return in_maps


def kernel(**inputs):
    in_maps = prepare_in_maps(**inputs)
    nc, out_name = _get_program()
    res = run_bass_kernel_spmd(nc, in_maps, core_ids=list(range(N_CORES)))
    acc = np.zeros((T_NOISE, D), dtype=np.float32)
    for r in res.results:
        acc += r[out_name].astype(np.float32)
    return acc


def run_traced(inputs, **kw):
    """Run once with NTFF tracing; returns BassKernelResults (exec_time_ns)."""
    in_maps = prepare_in_maps(**inputs)
    nc, out_name = _get_program()
    return run_bass_kernel_spmd(nc, in_maps, core_ids=list(range(N_CORES)),
                                trace=True, **kw)
